# revision 20
# baseline (speedup 1.0000x reference)
"""Trainium2 Bass kernel for the Attention2 module.

Computation (per batch row b):
    att_h  = h[b] @ W_h.T + b_h                      # [A]
    dot    = tanh(p_att_feats[b] + att_h)            # [L, A]
    scores = dot @ W_a[0]  (+ b_a, dropped: softmax shift-invariant)
    scores = where(mask, -1e8, scores)
    w      = softmax(scores)                         # [L]
    out[b] = w @ att_feats[b]                        # [R]

Sharding: data-parallel over batch B=32 across 8 cores (4 rows/core).

Per-core mapping (L=2048 -> 16 chunks of 128 partitions):
  phase 0: att_h for the 4 local rows via PE (K=RNN on partitions),
           +b_h on DVE, then partition-broadcast via a DRAM bounce.
  phase A: p-tile [128(l), 512(a)]; DVE add of broadcast att_h; ACT tanh
           (in place); DVE tensor_tensor_reduce fuses the W_a multiply and
           the A-axis sum -> scores column [128, 1].
  softmax: no max subtraction (|scores| <= ~23 so exp can't overflow);
           ACT exp, mask applied multiplicatively (keep in {0,1}) fused
           with the row-sum via tensor_tensor_reduce; partition sum via a
           ones-vector PE matmul; reciprocal on DVE.  exp(-1e8) == 0 in
           the reference, identical to multiplying exp(s) by 0.
  phase B: out[b] = sum_l w[l] * att_feats[b,l,:] as PE matmuls:
           lhsT = w column [128(l), 1], rhs = f-tile [128(l), 512(r)],
           accumulated over the 16 l-chunks into PSUM [1, 512] x 2.
           float32r (full-rate fp32 matmul mode) on both operands.
  scale:   result * (1/Z) on DVE, DMA out.

Host-side prep is layout only: transposes of h/W_h (so the contraction
dim lands on partitions with unit-stride loads) and the boolean mask
converted to a float keep-mask in score layout.
"""

import sys

import numpy as np

sys.path.insert(0, "/opt/trn_rl_repo")

import concourse.bass as bass  # noqa: E402
import concourse.tile as tile  # noqa: E402
from concourse import bacc, mybir  # noqa: E402
from concourse.bass_utils import run_bass_kernel_spmd  # noqa: E402

N_CORES = 8
B, L, RNN, A = 32, 2048, 1024, 512
BS = B // N_CORES

F32 = mybir.dt.float32
F32R = mybir.dt.float32r
MULT = mybir.AluOpType.mult
ADD = mybir.AluOpType.add
TANH = mybir.ActivationFunctionType.Tanh
EXP = mybir.ActivationFunctionType.Exp


KERNEL_VERSION = 5


def build_program(bs=BS, ll=L, rnn=RNN, a=A, lgrp=4, use_f32r=False,
                  p_bufs=3, f_bufs=4):
    nch = ll // 128          # l-chunks of 128 partitions
    ng = nch // lgrp         # DMA groups (lgrp l-chunks per transfer)
    kch = rnn // 128         # contraction chunks for att_h
    nh = (rnn + 511) // 512  # 512-wide output halves of phase B
    rh = rnn // nh           # free width per output half

    fdt = F32R if use_f32r else F32
    nc = bacc.Bacc(None, target_bir_lowering=False)
    p = nc.dram_tensor("p", [bs, ll, a], F32, kind="ExternalInput")
    f = nc.dram_tensor("f", [bs, ll, rnn], fdt, kind="ExternalInput")
    hT = nc.dram_tensor("hT", [rnn, bs], F32, kind="ExternalInput")
    whT = nc.dram_tensor("whT", [rnn, a], F32, kind="ExternalInput")
    bh = nc.dram_tensor("bh", [1, a], F32, kind="ExternalInput")
    wa = nc.dram_tensor("wa", [1, a], F32, kind="ExternalInput")
    keep = nc.dram_tensor("keep", [bs, 128, nch], F32, kind="ExternalInput")
    # unused input whose SHAPE encodes the kernel version: the compile
    # cache keys on the HLO signature (names/shapes), NOT the embedded
    # BIR — without this, a rebuilt kernel with unchanged I/O silently
    # re-runs the previously cached NEFF.
    ver = nc.dram_tensor("ver", [KERNEL_VERSION, 1], F32,
                         kind="ExternalInput")
    out = nc.dram_tensor("out", [bs, rnn], F32, kind="ExternalOutput")

    pr = p[:, :, :].rearrange("b (n q) a -> b q n a", q=128)
    fr = f[:, :, :].rearrange("b (n q) r -> b q n r", q=128)
    hTr = hT[:, :].rearrange("(c q) b -> q c b", q=128)
    whTr = whT[:, :].rearrange("(c q) a -> q c a", q=128)
    keepr = keep[:, :, :].rearrange("b q n -> q b n")

    with tile.TileContext(nc) as tc:
        with (
            tc.tile_pool(name="singles", bufs=1) as singles,
            tc.tile_pool(name="ppool", bufs=p_bufs) as ppool,
            tc.tile_pool(name="fpool", bufs=f_bufs) as fpool,
            tc.tile_pool(name="sm", bufs=3) as smpool,
            tc.tile_pool(name="respool", bufs=2) as respool,
            tc.tile_pool(name="psacc", bufs=2, space="PSUM") as psacc,
            tc.tile_pool(name="pssmall", bufs=2, space="PSUM") as pssmall,
        ):
            # ---- constants ----
            hT_sb = singles.tile([128, kch, bs], F32)
            nc.sync.dma_start(out=hT_sb, in_=hTr)
            whT_sb = singles.tile([128, kch, a], F32)
            nc.sync.dma_start(out=whT_sb, in_=whTr)
            # plain single-row loads (partition 0)
            bh_row = singles.tile([1, a], F32)
            nc.sync.dma_start(out=bh_row, in_=bh[:, :])
            wa_row = singles.tile([1, a], F32)
            nc.sync.dma_start(out=wa_row, in_=wa[:, :])
            keep_sb = singles.tile([128, bs, nch], F32)
            nc.sync.dma_start(out=keep_sb, in_=keepr)
            ver_sb = singles.tile([KERNEL_VERSION, 1], F32)
            nc.sync.dma_start(out=ver_sb, in_=ver[:, :])
            ones_sb = singles.tile([128, 1], F32)
            nc.vector.memset(ones_sb, 1.0)
            # ones row for K=1 partition-broadcast matmuls
            ones_row = singles.tile([1, 128], F32)
            nc.vector.memset(ones_row, 1.0)
            wa_sb = singles.tile([128, a], F32)
            attb_bc = singles.tile([128, bs, a], F32)

            # ---- phase 0: attb[b] = h[b] @ W_h.T + b_h, then broadcast
            # across all 128 partitions via a K=1 ones-matmul (avoids
            # irregular 0-stride broadcast DMAs entirely).
            with tc.tile_pool(name="ps0", bufs=1, space="PSUM") as ps0:
                wa_ps = ps0.tile([128, a], F32, tag="bc")
                nc.tensor.matmul(wa_ps, lhsT=ones_row, rhs=wa_row,
                                 start=True, stop=True)
                nc.scalar.copy(out=wa_sb, in_=wa_ps)
                for b in range(bs):
                    ah_ps = ps0.tile([1, a], F32, tag="ah", name=f"ah{b}")
                    for c in range(kch):
                        nc.tensor.matmul(ah_ps, lhsT=hT_sb[:, c, b:b + 1],
                                         rhs=whT_sb[:, c, :],
                                         start=(c == 0), stop=(c == kch - 1))
                    attb_row = smpool.tile([1, a], F32, tag="attbrow")
                    nc.vector.tensor_add(attb_row, ah_ps, bh_row)
                    bc_ps = ps0.tile([128, a], F32, tag="bc", name=f"bc{b}")
                    nc.tensor.matmul(bc_ps, lhsT=ones_row, rhs=attb_row,
                                     start=True, stop=True)
                    nc.scalar.copy(out=attb_bc[:, b, :], in_=bc_ps)

            for b in range(bs):
                # ---- phase A: scores[l] = W_a . tanh(p[l] + attb) ----
                scores = smpool.tile([128, nch], F32, tag="scores")
                for t in range(ng):
                    ptile = ppool.tile([128, lgrp, a], F32, tag="p")
                    nc.sync.dma_start(
                        out=ptile, in_=pr[b, :, t * lgrp:(t + 1) * lgrp, :])
                    for j in range(lgrp):
                        nc.vector.tensor_add(
                            ptile[:, j, :], ptile[:, j, :], attb_bc[:, b, :])
                    nc.scalar.activation(out=ptile, in_=ptile, func=TANH)
                    for j in range(lgrp):
                        i = t * lgrp + j
                        # multiply by W_a on DVE, free-dim sum via the
                        # reduce primitive (tensor_tensor_reduce crashes
                        # execution on this runtime)
                        nc.vector.tensor_mul(
                            ptile[:, j, :], ptile[:, j, :], wa_sb)
                        nc.vector.reduce_sum(
                            scores[:, i:i + 1], ptile[:, j, :],
                            axis=mybir.AxisListType.X)

                # ---- softmax weights (no max subtraction needed) ----
                e_sb = smpool.tile([128, nch], F32, tag="e")
                nc.scalar.activation(out=e_sb, in_=scores, func=EXP)
                w_sb = smpool.tile([128, nch], fdt, tag="w")
                nc.vector.tensor_mul(w_sb, e_sb, keep_sb[:, b, :])
                zpart = smpool.tile([128, 1], F32, tag="zpart")
                nc.vector.reduce_sum(zpart, w_sb, axis=mybir.AxisListType.X)
                z_ps = pssmall.tile([1, 1], F32, tag="zps")
                nc.tensor.matmul(z_ps, lhsT=ones_sb, rhs=zpart,
                                 start=True, stop=True)
                zinv = smpool.tile([1, 1], F32, tag="zinv")
                nc.vector.reciprocal(zinv, z_ps)

                # ---- phase B: out[b] = (w/Z) @ att_feats[b] ----
                rps = [psacc.tile([1, rh], F32, tag=f"r{hh}", name=f"rps{hh}")
                       for hh in range(nh)]
                for t in range(ng):
                    ftile = fpool.tile([128, lgrp, rnn], fdt, tag="f")
                    nc.sync.dma_start(
                        out=ftile, in_=fr[b, :, t * lgrp:(t + 1) * lgrp, :])
                    for j in range(lgrp):
                        i = t * lgrp + j
                        lhs = w_sb[:, i:i + 1]
                        for hh in range(nh):
                            nc.tensor.matmul(
                                rps[hh], lhsT=lhs,
                                rhs=ftile[:, j, hh * rh:(hh + 1) * rh],
                                start=(i == 0), stop=(i == nch - 1))
                res = respool.tile([1, rnn], F32, tag="res")
                for hh in range(nh):
                    nc.vector.tensor_scalar_mul(
                        res[:, hh * rh:(hh + 1) * rh], rps[hh], zinv)
                nc.sync.dma_start(out=out[b:b + 1, :], in_=res)
    nc.finalize()
    return nc


_PROG = None


def _get_program():
    global _PROG
    if _PROG is None:
        _PROG = build_program()
    return _PROG


def make_in_maps(h, att_feats, p_att_feats, mask, W_h, b_h, W_a):
    h = np.ascontiguousarray(np.asarray(h, dtype=np.float32))
    att_feats = np.asarray(att_feats, dtype=np.float32)
    p_att_feats = np.asarray(p_att_feats, dtype=np.float32)
    mask = np.asarray(mask)

    hT = np.ascontiguousarray(h.T)                                 # [RNN, B]
    whT = np.ascontiguousarray(np.asarray(W_h, np.float32).T)      # [RNN, A]
    bh = np.ascontiguousarray(np.asarray(b_h, np.float32).reshape(1, A))
    wa = np.ascontiguousarray(np.asarray(W_a, np.float32).reshape(1, A))
    # keep[b, p, i] = 1 - mask[b, i*128 + p]  (score-layout keep mask)
    keep = np.ascontiguousarray(
        (~mask).astype(np.float32).reshape(B, L // 128, 128).transpose(0, 2, 1))

    ver = np.zeros((KERNEL_VERSION, 1), np.float32)
    in_maps = []
    for c in range(N_CORES):
        s = slice(c * BS, (c + 1) * BS)
        in_maps.append({
            "p": np.ascontiguousarray(p_att_feats[s]),
            "f": np.ascontiguousarray(att_feats[s]),
            "hT": np.ascontiguousarray(hT[:, s]),
            "whT": whT,
            "bh": bh,
            "wa": wa,
            "keep": np.ascontiguousarray(keep[s]),
            "ver": ver,
        })
    return in_maps


def run_sharded(inputs, trace=False, **kwargs):
    nc = _get_program()
    in_maps = make_in_maps(
        inputs["h"], inputs["att_feats"], inputs["p_att_feats"],
        inputs["mask"], inputs["W_h"], inputs["b_h"], inputs["W_a"])
    return run_bass_kernel_spmd(nc, in_maps, core_ids=list(range(N_CORES)),
                                trace=trace, **kwargs)


def kernel(h, att_feats, p_att_feats, mask, W_h, b_h, W_a, b_a):
    res = run_sharded({
        "h": h, "att_feats": att_feats, "p_att_feats": p_att_feats,
        "mask": mask, "W_h": W_h, "b_h": b_h, "W_a": W_a, "b_a": b_a})
    return np.concatenate([res.results[c]["out"] for c in range(N_CORES)],
                          axis=0).astype(np.float32)


# revision 21
# speedup vs baseline: 1.1238x; 1.1238x over previous
"""Trainium2 Bass kernel for the Attention2 module.

Computation (per batch row b):
    att_h  = h[b] @ W_h.T + b_h                      # [A]
    dot    = tanh(p_att_feats[b] + att_h)            # [L, A]
    scores = dot @ W_a[0]  (+ b_a, dropped: softmax shift-invariant)
    scores = where(mask, -1e8, scores)
    w      = softmax(scores)                         # [L]
    out[b] = w @ att_feats[b]                        # [R]

Sharding: data-parallel over batch B=32 across 8 cores (4 rows/core).

Per-core mapping (L=2048 -> 16 chunks of 128 partitions):
  phase 0: att_h for the 4 local rows via PE (K=RNN on partitions),
           +b_h on DVE, then partition-broadcast via a DRAM bounce.
  phase A: p-tile [128(l), 512(a)]; DVE add of broadcast att_h; ACT tanh
           (in place); DVE tensor_tensor_reduce fuses the W_a multiply and
           the A-axis sum -> scores column [128, 1].
  softmax: no max subtraction (|scores| <= ~23 so exp can't overflow);
           ACT exp, mask applied multiplicatively (keep in {0,1}) fused
           with the row-sum via tensor_tensor_reduce; partition sum via a
           ones-vector PE matmul; reciprocal on DVE.  exp(-1e8) == 0 in
           the reference, identical to multiplying exp(s) by 0.
  phase B: out[b] = sum_l w[l] * att_feats[b,l,:] as PE matmuls:
           lhsT = w column [128(l), 1], rhs = f-tile [128(l), 512(r)],
           accumulated over the 16 l-chunks into PSUM [1, 512] x 2.
           float32r (full-rate fp32 matmul mode) on both operands.
  scale:   result * (1/Z) on DVE, DMA out.

Host-side prep is layout only: transposes of h/W_h (so the contraction
dim lands on partitions with unit-stride loads) and the boolean mask
converted to a float keep-mask in score layout.
"""

import sys

import numpy as np

sys.path.insert(0, "/opt/trn_rl_repo")

import concourse.bass as bass  # noqa: E402
import concourse.tile as tile  # noqa: E402
from concourse import bacc, mybir  # noqa: E402
from concourse.bass_utils import run_bass_kernel_spmd  # noqa: E402

N_CORES = 8
B, L, RNN, A = 32, 2048, 1024, 512
BS = B // N_CORES

F32 = mybir.dt.float32
F32R = mybir.dt.float32r
MULT = mybir.AluOpType.mult
ADD = mybir.AluOpType.add
TANH = mybir.ActivationFunctionType.Tanh
EXP = mybir.ActivationFunctionType.Exp


KERNEL_VERSION = 6


def build_program(bs=BS, ll=L, rnn=RNN, a=A, lgrp=4, use_f32r=True,
                  p_bufs=3, f_bufs=4):
    nch = ll // 128          # l-chunks of 128 partitions
    ng = nch // lgrp         # DMA groups (lgrp l-chunks per transfer)
    kch = rnn // 128         # contraction chunks for att_h
    nh = (rnn + 511) // 512  # 512-wide output halves of phase B
    rh = rnn // nh           # free width per output half

    fdt = F32R if use_f32r else F32
    nc = bacc.Bacc(None, target_bir_lowering=False)
    p = nc.dram_tensor("p", [bs, ll, a], F32, kind="ExternalInput")
    f = nc.dram_tensor("f", [bs, ll, rnn], fdt, kind="ExternalInput")
    hT = nc.dram_tensor("hT", [rnn, bs], F32, kind="ExternalInput")
    whT = nc.dram_tensor("whT", [rnn, a], F32, kind="ExternalInput")
    bh = nc.dram_tensor("bh", [1, a], F32, kind="ExternalInput")
    wa = nc.dram_tensor("wa", [1, a], F32, kind="ExternalInput")
    keep = nc.dram_tensor("keep", [bs, 128, nch], F32, kind="ExternalInput")
    # unused input whose SHAPE encodes the kernel version: the compile
    # cache keys on the HLO signature (names/shapes), NOT the embedded
    # BIR — without this, a rebuilt kernel with unchanged I/O silently
    # re-runs the previously cached NEFF.
    ver = nc.dram_tensor("ver", [KERNEL_VERSION, 1], F32,
                         kind="ExternalInput")
    out = nc.dram_tensor("out", [bs, rnn], F32, kind="ExternalOutput")

    pr = p[:, :, :].rearrange("b (n q) a -> b q n a", q=128)
    fr = f[:, :, :].rearrange("b (n q) r -> b q n r", q=128)
    hTr = hT[:, :].rearrange("(c q) b -> q c b", q=128)
    whTr = whT[:, :].rearrange("(c q) a -> q c a", q=128)
    keepr = keep[:, :, :].rearrange("b q n -> q b n")

    with tile.TileContext(nc) as tc:
        with (
            tc.tile_pool(name="singles", bufs=1) as singles,
            tc.tile_pool(name="ppool", bufs=p_bufs) as ppool,
            tc.tile_pool(name="fpool", bufs=f_bufs) as fpool,
            tc.tile_pool(name="sm", bufs=3) as smpool,
            tc.tile_pool(name="respool", bufs=2) as respool,
            tc.tile_pool(name="psacc", bufs=2, space="PSUM") as psacc,
            tc.tile_pool(name="pssmall", bufs=2, space="PSUM") as pssmall,
        ):
            # ---- constants ----
            hT_sb = singles.tile([128, kch, bs], F32)
            nc.sync.dma_start(out=hT_sb, in_=hTr)
            whT_sb = singles.tile([128, kch, a], F32)
            nc.sync.dma_start(out=whT_sb, in_=whTr)
            # plain single-row loads (partition 0)
            bh_row = singles.tile([1, a], F32)
            nc.sync.dma_start(out=bh_row, in_=bh[:, :])
            wa_row = singles.tile([1, a], F32)
            nc.sync.dma_start(out=wa_row, in_=wa[:, :])
            keep_sb = singles.tile([128, bs, nch], F32)
            nc.sync.dma_start(out=keep_sb, in_=keepr)
            ver_sb = singles.tile([KERNEL_VERSION, 1], F32)
            nc.sync.dma_start(out=ver_sb, in_=ver[:, :])
            ones_sb = singles.tile([128, 1], F32)
            nc.vector.memset(ones_sb, 1.0)
            # ones row for K=1 partition-broadcast matmuls
            ones_row = singles.tile([1, 128], F32)
            nc.vector.memset(ones_row, 1.0)
            wa_sb = singles.tile([128, a], F32)
            attb_bc = singles.tile([128, bs, a], F32)

            # ---- phase 0: attb[b] = h[b] @ W_h.T + b_h, then broadcast
            # across all 128 partitions via a K=1 ones-matmul (avoids
            # irregular 0-stride broadcast DMAs entirely).
            with tc.tile_pool(name="ps0", bufs=1, space="PSUM") as ps0:
                wa_ps = ps0.tile([128, a], F32, tag="bc")
                nc.tensor.matmul(wa_ps, lhsT=ones_row, rhs=wa_row,
                                 start=True, stop=True)
                nc.scalar.copy(out=wa_sb, in_=wa_ps)
                for b in range(bs):
                    ah_ps = ps0.tile([1, a], F32, tag="ah", name=f"ah{b}")
                    for c in range(kch):
                        nc.tensor.matmul(ah_ps, lhsT=hT_sb[:, c, b:b + 1],
                                         rhs=whT_sb[:, c, :],
                                         start=(c == 0), stop=(c == kch - 1))
                    attb_row = smpool.tile([1, a], F32, tag="attbrow")
                    nc.vector.tensor_add(attb_row, ah_ps, bh_row)
                    bc_ps = ps0.tile([128, a], F32, tag="bc", name=f"bc{b}")
                    nc.tensor.matmul(bc_ps, lhsT=ones_row, rhs=attb_row,
                                     start=True, stop=True)
                    nc.scalar.copy(out=attb_bc[:, b, :], in_=bc_ps)

            for b in range(bs):
                # ---- phase A: scores[l] = W_a . tanh(p[l] + attb) ----
                scores = smpool.tile([128, nch], F32, tag="scores")
                for t in range(ng):
                    ptile = ppool.tile([128, lgrp, a], F32, tag="p")
                    nc.sync.dma_start(
                        out=ptile, in_=pr[b, :, t * lgrp:(t + 1) * lgrp, :])
                    for j in range(lgrp):
                        nc.vector.tensor_add(
                            ptile[:, j, :], ptile[:, j, :], attb_bc[:, b, :])
                    nc.scalar.activation(out=ptile, in_=ptile, func=TANH)
                    for j in range(lgrp):
                        i = t * lgrp + j
                        # multiply by W_a on DVE, free-dim sum via the
                        # reduce primitive (tensor_tensor_reduce crashes
                        # execution on this runtime)
                        nc.vector.tensor_mul(
                            ptile[:, j, :], ptile[:, j, :], wa_sb)
                        nc.vector.reduce_sum(
                            scores[:, i:i + 1], ptile[:, j, :],
                            axis=mybir.AxisListType.X)

                # ---- softmax weights (no max subtraction needed) ----
                e_sb = smpool.tile([128, nch], F32, tag="e")
                nc.scalar.activation(out=e_sb, in_=scores, func=EXP)
                w_sb = smpool.tile([128, nch], fdt, tag="w")
                nc.vector.tensor_mul(w_sb, e_sb, keep_sb[:, b, :])
                zpart = smpool.tile([128, 1], F32, tag="zpart")
                nc.vector.reduce_sum(zpart, w_sb, axis=mybir.AxisListType.X)
                z_ps = pssmall.tile([1, 1], F32, tag="zps")
                nc.tensor.matmul(z_ps, lhsT=ones_sb, rhs=zpart,
                                 start=True, stop=True)
                zinv = smpool.tile([1, 1], F32, tag="zinv")
                nc.vector.reciprocal(zinv, z_ps)

                # ---- phase B: out[b] = (w/Z) @ att_feats[b] ----
                rps = [psacc.tile([1, rh], F32, tag=f"r{hh}", name=f"rps{hh}")
                       for hh in range(nh)]
                for t in range(ng):
                    ftile = fpool.tile([128, lgrp, rnn], fdt, tag="f")
                    nc.sync.dma_start(
                        out=ftile, in_=fr[b, :, t * lgrp:(t + 1) * lgrp, :])
                    for j in range(lgrp):
                        i = t * lgrp + j
                        lhs = w_sb[:, i:i + 1]
                        for hh in range(nh):
                            nc.tensor.matmul(
                                rps[hh], lhsT=lhs,
                                rhs=ftile[:, j, hh * rh:(hh + 1) * rh],
                                start=(i == 0), stop=(i == nch - 1))
                res = respool.tile([1, rnn], F32, tag="res")
                for hh in range(nh):
                    nc.vector.tensor_scalar_mul(
                        res[:, hh * rh:(hh + 1) * rh], rps[hh], zinv)
                nc.sync.dma_start(out=out[b:b + 1, :], in_=res)
    nc.finalize()
    return nc


_PROG = None


def _get_program():
    global _PROG
    if _PROG is None:
        _PROG = build_program()
    return _PROG


def make_in_maps(h, att_feats, p_att_feats, mask, W_h, b_h, W_a):
    h = np.ascontiguousarray(np.asarray(h, dtype=np.float32))
    att_feats = np.asarray(att_feats, dtype=np.float32)
    p_att_feats = np.asarray(p_att_feats, dtype=np.float32)
    mask = np.asarray(mask)

    hT = np.ascontiguousarray(h.T)                                 # [RNN, B]
    whT = np.ascontiguousarray(np.asarray(W_h, np.float32).T)      # [RNN, A]
    bh = np.ascontiguousarray(np.asarray(b_h, np.float32).reshape(1, A))
    wa = np.ascontiguousarray(np.asarray(W_a, np.float32).reshape(1, A))
    # keep[b, p, i] = 1 - mask[b, i*128 + p]  (score-layout keep mask)
    keep = np.ascontiguousarray(
        (~mask).astype(np.float32).reshape(B, L // 128, 128).transpose(0, 2, 1))

    ver = np.zeros((KERNEL_VERSION, 1), np.float32)
    in_maps = []
    for c in range(N_CORES):
        s = slice(c * BS, (c + 1) * BS)
        in_maps.append({
            "p": np.ascontiguousarray(p_att_feats[s]),
            "f": np.ascontiguousarray(att_feats[s]),
            "hT": np.ascontiguousarray(hT[:, s]),
            "whT": whT,
            "bh": bh,
            "wa": wa,
            "keep": np.ascontiguousarray(keep[s]),
            "ver": ver,
        })
    return in_maps


def run_sharded(inputs, trace=False, **kwargs):
    nc = _get_program()
    in_maps = make_in_maps(
        inputs["h"], inputs["att_feats"], inputs["p_att_feats"],
        inputs["mask"], inputs["W_h"], inputs["b_h"], inputs["W_a"])
    return run_bass_kernel_spmd(nc, in_maps, core_ids=list(range(N_CORES)),
                                trace=trace, **kwargs)


def kernel(h, att_feats, p_att_feats, mask, W_h, b_h, W_a, b_a):
    res = run_sharded({
        "h": h, "att_feats": att_feats, "p_att_feats": p_att_feats,
        "mask": mask, "W_h": W_h, "b_h": b_h, "W_a": W_a, "b_a": b_a})
    return np.concatenate([res.results[c]["out"] for c in range(N_CORES)],
                          axis=0).astype(np.float32)
